# revision 12
# baseline (speedup 1.0000x reference)
"""Bass/Tile kernel for nn_CTransformer (3x3 neighborhood attention), TRN2.

Per-core layout: channel-on-partition. Core handles 32 image rows of one batch
(+1 halo row each side), width padded to 66. Tokens NT = 34*66 = 2244.

v3 design notes:
  - strip-outer pipeline: stats -> LN -> xhat -> qkv per 374-token strip, so
    DMA, stats math and qkv matmuls overlap instead of running as serial
    phases.
  - x input DMA split across both HWDGE queues (sync + scalar).
  - qkv = 3 matmul streams per (dblock, strip): 2 weight halves + one rank-2
    stream carrying the mean correction (-wsum x murstd) and the bias.
  - LN rstd / softmax denominator via reciprocal_approx_fast.
  - erep (attention-weight replication 72 -> 128 rows) via broadcast DMA on
    the HWDGE queues instead of PE matmul + psum evac.
  - q/k/v stored as [128, 2, NT]; each q*k product is one tensor_tensor op
    over both halves (FD=1024, 2x mode).
  - out-proj bias + residual folded into one scalar_tensor_tensor evac.
"""
import numpy as np
from contextlib import ExitStack

try:
    import concourse.bass as bass
except ImportError:
    import sys
    sys.path.insert(0, '/opt/trn_rl_repo')
    import concourse.bass as bass
import concourse.bacc as bacc
import concourse.tile as tile
from concourse import mybir

F32 = mybir.dt.float32
BF16 = mybir.dt.bfloat16

B, C, H, W = 4, 256, 64, 64
NHEAD, DH = 8, 32
EPS = 1e-5
NCORE = 8
RPC = 32                 # image rows per core
R2, W2 = RPC + 2, W + 2  # 34 x 66 padded grid
NT = R2 * W2             # 2244
SS = 374                 # qkv token strip (6 strips)
NS = NT // SS            # 6
CS = 512                 # core-token strip (8 image rows)
NCS = 4                  # 4 core strips
OFFS = [(i, j) for i in range(3) for j in range(3)]
AD = BF16
EREP_DMA = False          # replicate attn rows via broadcast DMA (vs PE matmul)

# repack-16 permutation: qkv row (chunk*128 + h*16 + d) <- channel (h*32 + chunk*16 + d)
PERM = np.zeros(C, dtype=np.int64)
for _k in range(2):
    for _h in range(8):
        for _d in range(16):
            PERM[_k * 128 + _h * 16 + _d] = _h * 32 + _k * 16 + _d


def host_prep(inputs):
    """Fold LN/scale/permutation into weights; build constant matrices."""
    f = np.float32
    ln_w = np.asarray(inputs['ln_w'], np.float64)
    ln_b = np.asarray(inputs['ln_b'], np.float64)
    ipw = np.asarray(inputs['in_proj_w'], np.float64)
    ipb = np.asarray(inputs['in_proj_b'], np.float64)
    opw = np.asarray(inputs['out_proj_w'], np.float64)
    opb = np.asarray(inputs['out_proj_b'], np.float64)
    scale = DH ** -0.5
    Wt = ipw * ln_w[None, :]
    bt = ipb + ipw @ ln_b
    Wt[:C] *= scale
    bt[:C] *= scale
    Wall = np.concatenate([Wt[i * C:(i + 1) * C][PERM] for i in range(3)], 0)  # [768,256]
    ball = np.concatenate([bt[i * C:(i + 1) * C][PERM] for i in range(3)], 0)
    wsum = Wall.sum(1)
    OWp = opw[:, PERM]

    import concourse.mybir as _mb
    bf = _mb.dt.np(_mb.dt.bfloat16)
    consts = {}
    consts['wt'] = np.ascontiguousarray(Wall.T).astype(bf)         # [256, 768] lhsT
    consts['rk2'] = np.stack([-wsum, ball]).astype(bf)             # [2, 768] lhsT
    consts['owt'] = np.ascontiguousarray(OWp.T).astype(bf)         # [256, 256] lhsT
    consts['obc'] = opb.astype(f).reshape(2, 128, 1)               # [2][128,1] bias cols
    consts['ones1'] = np.ones((128, 1), bf)
    consts['onesr'] = np.ones((1, 128), bf)
    consts['onesnt'] = np.ones((1, NT), bf)
    consts['epsb'] = np.full((3, 1), EPS, f)
    # scores masks: for offset t, [128, 72] with row r -> col t*8 + r//16
    mk = np.zeros((128, 9 * 72), f)
    for t in range(9):
        mk[np.arange(128), t * 72 + t * 8 + np.arange(128) // 16] = 1
    consts['mk'] = mk.astype(bf)
    # den sum: [72, 8]
    tm = np.zeros((72, 8), f)
    for t in range(9):
        for h in range(8):
            tm[t * 8 + h, h] = 1
    consts['tm'] = tm.astype(bf)
    # attn replication: [72, 9*128]
    rp = np.zeros((72, 9 * 128), f)
    for t in range(9):
        for c in range(128):
            rp[t * 8 + c // 16, t * 128 + c] = 1
    consts['rp'] = rp.astype(bf)
    # rden -> 128 rows: [8, 128] (head h -> rows 16h..16h+15)
    r128 = np.zeros((8, 128), f)
    for h in range(8):
        r128[h, 16 * h:16 * (h + 1)] = 1
    consts['r128'] = r128.astype(bf)
    consts['ident'] = np.eye(128, dtype=f).astype(bf)
    return consts


def core_inputs(inputs, consts):
    """Build per-core in_maps (pad + slice on host)."""
    src = np.asarray(inputs['src'], np.float32)
    maps = []
    for core in range(NCORE):
        b = core // 2
        r0 = (core % 2) * RPC
        xp = np.zeros((C, R2, W2), np.float32)
        rlo, rhi = r0 - 1, r0 + RPC + 1
        slo, shi = max(rlo, 0), min(rhi, H)
        xp[:, (slo - rlo):(shi - rlo), 1:W + 1] = src[b, :, slo:shi, :]
        m = {'x': xp.reshape(C, NT)}
        m.update(consts)
        maps.append(m)
    return maps


def build(erep_dma=EREP_DMA):
    nc = bacc.Bacc('TRN2', target_bir_lowering=False, debug=False, num_devices=NCORE)

    x_d = nc.dram_tensor('x', [C, NT], F32, kind='ExternalInput')
    wt_d = nc.dram_tensor('wt', [C, 3 * C], BF16, kind='ExternalInput')
    rk2_d = nc.dram_tensor('rk2', [2, 3 * C], BF16, kind='ExternalInput')
    owt_d = nc.dram_tensor('owt', [C, C], BF16, kind='ExternalInput')
    obc_d = nc.dram_tensor('obc', [2, 128, 1], F32, kind='ExternalInput')
    ones1_d = nc.dram_tensor('ones1', [128, 1], BF16, kind='ExternalInput')
    onesr_d = nc.dram_tensor('onesr', [1, 128], BF16, kind='ExternalInput')
    onesnt_d = nc.dram_tensor('onesnt', [1, NT], BF16, kind='ExternalInput')
    epsb_d = nc.dram_tensor('epsb', [3, 1], F32, kind='ExternalInput')
    mk_d = nc.dram_tensor('mk', [128, 9 * 72], BF16, kind='ExternalInput')
    tm_d = nc.dram_tensor('tm', [72, 8], BF16, kind='ExternalInput')
    rp_d = nc.dram_tensor('rp', [72, 9 * 128], BF16, kind='ExternalInput')
    r128_d = nc.dram_tensor('r128', [8, 128], BF16, kind='ExternalInput')
    id_d = nc.dram_tensor('ident', [128, 128], BF16, kind='ExternalInput')

    out_d = nc.dram_tensor('out', [C, RPC * W], F32, kind='ExternalOutput')

    with tile.TileContext(nc) as tc, ExitStack() as ctx:
        ctx.enter_context(nc.allow_low_precision(reason='bf16 attention path'))
        P = ctx.enter_context(tc.tile_pool(name='persist', bufs=1))
        T3 = ctx.enter_context(tc.tile_pool(name='work', bufs=3))
        PS = ctx.enter_context(tc.tile_pool(name='psum', bufs=1, space='PSUM'))

        mm = lambda *a, **kw: nc.tensor.matmul(*a, **kw)

        # ---- input x first (both queues), then constants ----
        xsb = [P.tile([128, NT], F32, tag=f'xsb{c}', name=f'xsb{c}') for c in range(2)]
        ones1_s = P.tile([128, 1], BF16, tag='ones1')
        epsb_s = P.tile([3, 1], F32, tag='epsb')
        for s in range(3):
            nc.sync.dma_start(xsb[0][:, bass.ts(s, 2 * SS)], x_d[0:128, bass.ts(s, 2 * SS)])
            nc.scalar.dma_start(xsb[1][:, bass.ts(s, 2 * SS)], x_d[128:256, bass.ts(s, 2 * SS)])
            if s == 0:
                nc.sync.dma_start(ones1_s[:], ones1_d[:])
                nc.scalar.dma_start(epsb_s[:], epsb_d[:])
        wt_c = [P.tile([128, 3 * C], BF16, tag=f'wt{c}', name=f'wt{c}') for c in range(2)]
        nc.sync.dma_start(wt_c[0][:], wt_d[0:128, :])
        nc.scalar.dma_start(wt_c[1][:], wt_d[128:256, :])
        rk2_s = P.tile([2, 3 * C], BF16, tag='rk2')
        nc.sync.dma_start(rk2_s[:], rk2_d[:])
        onesr_s = P.tile([1, 128], BF16, tag='onesr')
        nc.scalar.dma_start(onesr_s[:], onesr_d[:])
        owt_c = [P.tile([128, C], BF16, tag=f'owt{c}', name=f'owt{c}') for c in range(2)]
        nc.sync.dma_start(owt_c[0][:], owt_d[0:128, :])
        nc.scalar.dma_start(owt_c[1][:], owt_d[128:256, :])
        obc_s = [P.tile([128, 1], F32, tag=f'obc{c}', name=f'obc{c}') for c in range(2)]
        nc.sync.dma_start(obc_s[0][:], obc_d[0])
        nc.scalar.dma_start(obc_s[1][:], obc_d[1])
        mk_s = P.tile([128, 9 * 72], AD, tag='mk')
        nc.sync.dma_start(mk_s[:], mk_d[:])
        tm_s = P.tile([72, 8], AD, tag='tm')
        nc.scalar.dma_start(tm_s[:], tm_d[:])
        rp_s = P.tile([72, 9 * 128], AD, tag='rp')
        nc.sync.dma_start(rp_s[:], rp_d[:])
        r128_s = P.tile([8, 128], AD, tag='r128')
        nc.scalar.dma_start(r128_s[:], r128_d[:])
        id_s = P.tile([128, 128], AD, tag='ident')
        nc.sync.dma_start(id_s[:], id_d[:])
        r2 = P.tile([2, NT], BF16, tag='r2')
        nc.scalar.dma_start(r2[1:2, :], onesnt_d[:])
        rstd1 = P.tile([1, NT], BF16, tag='rstd1')

        xb16 = [P.tile([128, NT], BF16, tag=f'xb16{c}', name=f'xb16{c}') for c in range(2)]
        xhat = [P.tile([128, NT], BF16, tag=f'xhat{c}', name=f'xhat{c}') for c in range(2)]
        qkv = [P.tile([128, 2, NT], AD, tag=f'qkv{t}', name=f'qkv{t}') for t in range(3)]

        # ---- per-strip: stats -> (batched LN) -> xhat -> qkv ----
        evac_rr = [0]

        def evac(dst_ap, src_ap):
            if evac_rr[0] % 2 == 0:
                nc.scalar.activation(dst_ap, src_ap, mybir.ActivationFunctionType.Copy)
            else:
                nc.vector.tensor_copy(dst_ap, src_ap)
            evac_rr[0] += 1

        NB = 3  # strips per LN batch
        mu_rows = {}
        for s in range(NS):
            sl = bass.ts(s, SS)
            b = s // NB
            if s % NB == 0:
                mu_rows[b] = (T3.tile([NB, SS], F32, tag='mu6', name='mu6', bufs=2),
                              T3.tile([NB, SS], F32, tag='m26', name='m26', bufs=2))
            mub, m2b = mu_rows[b]
            xsq = [T3.tile([128, SS], BF16, tag='xsq', name='xsq', bufs=4) for _ in range(2)]
            for c in range(2):
                nc.vector.tensor_copy(xb16[c][:, sl], xsb[c][:, sl])
                nc.scalar.square(xsq[c][:], xsb[c][:, sl])
            ps_sx = PS.tile([1, SS], F32, tag='psB', bufs=2, name='ps_sx')
            ps_sxx = PS.tile([1, SS], F32, tag='psB', bufs=2, name='ps_sxx')
            for c in range(2):
                mm(ps_sx[:], ones1_s[:], xb16[c][:, sl], start=(c == 0), stop=(c == 1))
                mm(ps_sxx[:], ones1_s[:], xsq[c][:], start=(c == 0), stop=(c == 1))
            tmu = T3.tile([1, SS], F32, tag='txe', name='tmu', bufs=4)
            tm2 = T3.tile([1, SS], F32, tag='txe', name='tm2', bufs=4)
            nc.scalar.activation(tmu[:], ps_sx[:],
                                 mybir.ActivationFunctionType.Copy, scale=1.0 / C)
            nc.vector.tensor_copy(tm2[:], ps_sxx[:])
            nc.sync.dma_start(mub[s % NB:s % NB + 1, :], tmu[:])
            nc.scalar.dma_start(m2b[s % NB:s % NB + 1, :], tm2[:])

            if s % NB == NB - 1:
                # batched LN math for strips [b*NB, b*NB+NB)
                bl = bass.ts(b, NB * SS)
                musq = T3.tile([NB, SS], F32, tag='musq', bufs=2)
                nc.vector.tensor_mul(musq[:], mub[:], mub[:])
                var = T3.tile([NB, SS], F32, tag='var', bufs=2)
                # var = m2/C - musq
                nc.vector.scalar_tensor_tensor(var[:], m2b[:], 1.0 / C, musq[:],
                                               mybir.AluOpType.mult,
                                               mybir.AluOpType.subtract)
                sd = T3.tile([NB, SS], F32, tag='sd', bufs=2)
                nc.scalar.activation(sd[:], var[:], mybir.ActivationFunctionType.Sqrt,
                                     bias=epsb_s[:])
                rstd = T3.tile([NB, SS], F32, tag='rstd', bufs=2)
                nc.vector.reciprocal_approx_fast(rstd[:], sd[:])
                mrs = T3.tile([NB, SS], BF16, tag='mrs', bufs=2)
                nc.vector.tensor_mul(mrs[:], mub[:], rstd[:])
                rsb = T3.tile([NB, SS], BF16, tag='rsb', bufs=2)
                nc.vector.tensor_copy(rsb[:], rstd[:])
                nc.sync.dma_start(r2[0:1, bl], mrs[:])
                nc.scalar.dma_start(rstd1[:, bl], rsb[:])
                strips = list(range(b * NB, b * NB + NB))
                for s2 in strips:
                    sl2 = bass.ts(s2, SS)
                    ps_rb = PS.tile([128, SS], F32, tag='psO1', bufs=1, name='ps_rb')
                    mm(ps_rb[:], onesr_s[:], rstd1[:, sl2], start=True, stop=True)
                    rstdb = T3.tile([128, SS], BF16, tag='rstdb', name='rstdb', bufs=2)
                    nc.scalar.activation(rstdb[:], ps_rb[:],
                                         mybir.ActivationFunctionType.Copy)
                    for c in range(2):
                        nc.vector.tensor_mul(xhat[c][:, sl2], xb16[c][:, sl2], rstdb[:])
                # qkv: per dblock, run each lhsT across all 3 strips back-to-back
                for d in range(6):
                    dl = bass.ts(d, 128)
                    tens, half = qkv[d // 2], d % 2
                    pqs = {}
                    for k, s2 in enumerate(strips):
                        pqs[s2] = PS.tile([128, SS], F32,
                                          tag=('psA' if k % 2 == 0 else 'psC'),
                                          bufs=2, name='pq')
                    for c in range(2):
                        for s2 in strips:
                            mm(pqs[s2][:], wt_c[c][:, dl], xhat[c][:, bass.ts(s2, SS)],
                               start=(c == 0), stop=False)
                    for s2 in strips:
                        mm(pqs[s2][:], rk2_s[:, dl], r2[:, bass.ts(s2, SS)],
                           start=False, stop=True)
                    for s2 in strips:
                        evac(tens[:, half, bass.ts(s2, SS)], pqs[s2][:])

        # views [128, 2, 34, 66]
        g = lambda tn: tn[:].rearrange('p h (r w) -> p h r w', w=W2)
        qg, kg, vg = g(qkv[0]), g(qkv[1]), g(qkv[2])
        xg = [xsb[c][:].rearrange('p (r w) -> p r w', w=W2) for c in range(2)]

        # ---- attention per core strip ----
        for cs in range(NCS):
            r0 = 1 + 8 * cs
            ps_sc = PS.tile([72, CS], F32, tag='psB', bufs=2, name='ps_sc')
            for t, (i, j) in enumerate(OFFS):
                prod = T3.tile([128, 2, 8, W], AD, tag='prod', bufs=4, name='prod')
                nc.vector.tensor_mul(
                    prod[:],
                    qg[:, :, r0:r0 + 8, 1:1 + W],
                    kg[:, :, r0 + i - 1:r0 + i + 7, j:j + W])
                for c in range(2):
                    mm(ps_sc[:, :], mk_s[:, bass.ts(t, 72)], prod[:, c],
                       start=(t == 0 and c == 0), stop=(t == 8 and c == 1))
            e_sb = T3.tile([72, CS], AD, tag='e_sb', bufs=2)
            nc.scalar.activation(e_sb[:], ps_sc[:], mybir.ActivationFunctionType.Exp)
            # denominator path runs concurrent with the erep/avp rounds below;
            # normalization is applied at o evac time
            ps_den = PS.tile([8, CS], F32, tag='psA', bufs=2, name='ps_den')
            mm(ps_den[:], tm_s[:], e_sb[:], start=True, stop=True)
            rdenf = T3.tile([8, CS], F32, tag='rdenf', bufs=2)
            nc.vector.reciprocal_approx_fast(rdenf[:], ps_den[:])
            rden = T3.tile([8, CS], AD, tag='rden', bufs=2)
            nc.vector.tensor_copy(rden[:], rdenf[:])
            ps_r72 = PS.tile([128, CS], F32, tag='psA', bufs=2, name='ps_r72')
            mm(ps_r72[:], r128_s[:], rden[:], start=True, stop=True)
            rdrep = T3.tile([128, CS], AD, tag='rdrep', bufs=2)
            nc.scalar.activation(rdrep[:], ps_r72[:], mybir.ActivationFunctionType.Copy)
            attn = e_sb

            o_ps = [PS.tile([128, CS], F32, tag=f'psO{c}', bufs=1, name=f'o_ps{c}')
                    for c in range(2)]
            for t, (i, j) in enumerate(OFFS):
                erep = T3.tile([128, CS], AD, tag='erep', bufs=10)
                if erep_dma:
                    src = attn[t * 8:(t + 1) * 8, :].unsqueeze(1).broadcast_to(
                        (8, 16, CS))
                    dst = erep[:].rearrange('(h d) w -> h d w', d=16)
                    (nc.sync if t % 2 == 0 else nc.scalar).dma_start(dst, src)
                else:
                    ps_er = PS.tile([128, CS], F32, tag='psC', bufs=2, name='ps_er')
                    mm(ps_er[:], rp_s[:, bass.ts(t, 128)], attn[:],
                       start=True, stop=True)
                    nc.scalar.activation(erep[:], ps_er[:],
                                         mybir.ActivationFunctionType.Copy)
                avp = T3.tile([128, 2, 8, W], AD, tag='avp', bufs=4, name='avp')
                erv = erep[:].rearrange('p (r w) -> p r w', w=W)
                erv2 = erv.unsqueeze(1).broadcast_to((128, 2, 8, W))
                eng = nc.gpsimd if t == 4 else nc.vector
                eng.tensor_mul(
                    avp[:], erv2,
                    vg[:, :, r0 + i - 1:r0 + i + 7, j:j + W])
                for c in range(2):
                    mm(o_ps[c][:], id_s[:], avp[:, c], start=(t == 0), stop=(t == 8))
            o_sb = [T3.tile([128, CS], BF16, tag=f'o_sb{c}', name=f'o_sb{c}', bufs=2)
                    for c in range(2)]
            for c in range(2):
                nc.vector.tensor_mul(o_sb[c][:], o_ps[c][:], rdrep[:])

            # ---- out projection; bias + residual folded into evac ----
            for db in range(2):
                op_ps = PS.tile([128, CS], F32, tag='psA', bufs=2, name='op_ps')
                for c in range(2):
                    mm(op_ps[:], owt_c[c][:, bass.ts(db, 128)], o_sb[c][:],
                       start=(c == 0), stop=(c == 1))
                ot = T3.tile([128, 8, W], F32, tag='ot', bufs=2)
                nc.vector.scalar_tensor_tensor(
                    ot[:], op_ps[:].rearrange('p (r w) -> p r w', w=W),
                    obc_s[db][:], xg[db][:, r0:r0 + 8, 1:1 + W],
                    mybir.AluOpType.add, mybir.AluOpType.add)
                nc.sync.dma_start(
                    out_d[bass.ts(db, 128), bass.ts(cs, CS)], ot[:])

    nc.compile()
    return nc


_NC_CACHE = {}


def _get_nc(**kw):
    key = ('nc',) + tuple(sorted(kw.items()))
    if key not in _NC_CACHE:
        _NC_CACHE[key] = build(**kw)
    return _NC_CACHE[key]


def kernel(**inputs):
    """Full-input, full-output entry point. Shards over 8 NeuronCores."""
    from concourse.bass_utils import run_bass_kernel_spmd
    nc = _get_nc()
    consts = host_prep(inputs)
    maps = core_inputs(inputs, consts)
    res = run_bass_kernel_spmd(nc, maps, core_ids=list(range(NCORE)))
    out = np.zeros((B, C, H, W), np.float32)
    for core in range(NCORE):
        b = core // 2
        r0 = (core % 2) * RPC
        out[b, :, r0:r0 + RPC, :] = res.results[core]['out'].reshape(C, RPC, W)
    return out


# revision 13
# speedup vs baseline: 1.0757x; 1.0757x over previous
"""Bass/Tile kernel for nn_CTransformer (3x3 neighborhood attention), TRN2.

Per-core layout: channel-on-partition. Core handles 32 image rows of one batch
(+1 halo row each side), width padded to 66. Tokens NT = 34*66 = 2244.

v3 design notes:
  - strip-outer pipeline: stats -> LN -> xhat -> qkv per 374-token strip, so
    DMA, stats math and qkv matmuls overlap instead of running as serial
    phases.
  - x input DMA split across both HWDGE queues (sync + scalar).
  - qkv = 3 matmul streams per (dblock, strip): 2 weight halves + one rank-2
    stream carrying the mean correction (-wsum x murstd) and the bias.
  - LN rstd / softmax denominator via reciprocal_approx_fast.
  - erep (attention-weight replication 72 -> 128 rows) via broadcast DMA on
    the HWDGE queues instead of PE matmul + psum evac.
  - q/k/v stored as [128, 2, NT]; each q*k product is one tensor_tensor op
    over both halves (FD=1024, 2x mode).
  - out-proj bias + residual folded into one scalar_tensor_tensor evac.
"""
import numpy as np
from contextlib import ExitStack

try:
    import concourse.bass as bass
except ImportError:
    import sys
    sys.path.insert(0, '/opt/trn_rl_repo')
    import concourse.bass as bass
import concourse.bacc as bacc
import concourse.tile as tile
from concourse import mybir

F32 = mybir.dt.float32
BF16 = mybir.dt.bfloat16

B, C, H, W = 4, 256, 64, 64
NHEAD, DH = 8, 32
EPS = 1e-5
NCORE = 8
RPC = 32                 # image rows per core
R2, W2 = RPC + 2, W + 2  # 34 x 66 padded grid
NT = R2 * W2             # 2244
SS = 374                 # qkv token strip (6 strips)
NS = NT // SS            # 6
CS = 512                 # core-token strip (8 image rows)
NCS = 4                  # 4 core strips
OFFS = [(i, j) for i in range(3) for j in range(3)]
AD = BF16
EREP_DMA = False          # replicate attn rows via broadcast DMA (vs PE matmul)

# repack-16 permutation: qkv row (chunk*128 + h*16 + d) <- channel (h*32 + chunk*16 + d)
PERM = np.zeros(C, dtype=np.int64)
for _k in range(2):
    for _h in range(8):
        for _d in range(16):
            PERM[_k * 128 + _h * 16 + _d] = _h * 32 + _k * 16 + _d


def host_prep(inputs):
    """Fold LN/scale/permutation into weights; build constant matrices."""
    f = np.float32
    ln_w = np.asarray(inputs['ln_w'], np.float64)
    ln_b = np.asarray(inputs['ln_b'], np.float64)
    ipw = np.asarray(inputs['in_proj_w'], np.float64)
    ipb = np.asarray(inputs['in_proj_b'], np.float64)
    opw = np.asarray(inputs['out_proj_w'], np.float64)
    opb = np.asarray(inputs['out_proj_b'], np.float64)
    scale = DH ** -0.5
    Wt = ipw * ln_w[None, :]
    bt = ipb + ipw @ ln_b
    Wt[:C] *= scale
    bt[:C] *= scale
    Wall = np.concatenate([Wt[i * C:(i + 1) * C][PERM] for i in range(3)], 0)  # [768,256]
    ball = np.concatenate([bt[i * C:(i + 1) * C][PERM] for i in range(3)], 0)
    wsum = Wall.sum(1)
    OWp = opw[:, PERM]

    import concourse.mybir as _mb
    bf = _mb.dt.np(_mb.dt.bfloat16)
    consts = {}
    consts['wt'] = np.ascontiguousarray(Wall.T).astype(bf)         # [256, 768] lhsT
    consts['rk2'] = np.stack([-wsum, ball]).astype(bf)             # [2, 768] lhsT
    consts['owt'] = np.ascontiguousarray(OWp.T).astype(bf)         # [256, 256] lhsT
    consts['obc'] = opb.astype(f).reshape(2, 128, 1)               # [2][128,1] bias cols
    consts['ones1'] = np.ones((128, 1), bf)
    consts['onesr'] = np.ones((1, 128), bf)
    consts['onesnt'] = np.ones((1, NT), bf)
    consts['epsb'] = np.full((3, 1), EPS, f)
    # scores masks: for offset t, [128, 72] with row r -> col t*8 + r//16
    mk = np.zeros((128, 9 * 72), f)
    for t in range(9):
        mk[np.arange(128), t * 72 + t * 8 + np.arange(128) // 16] = 1
    consts['mk'] = mk.astype(bf)
    # den sum: [72, 8]
    tm = np.zeros((72, 8), f)
    for t in range(9):
        for h in range(8):
            tm[t * 8 + h, h] = 1
    consts['tm'] = tm.astype(bf)
    # attn replication: [72, 9*128]
    rp = np.zeros((72, 9 * 128), f)
    for t in range(9):
        for c in range(128):
            rp[t * 8 + c // 16, t * 128 + c] = 1
    consts['rp'] = rp.astype(bf)
    # rden -> 128 rows: [8, 128] (head h -> rows 16h..16h+15)
    r128 = np.zeros((8, 128), f)
    for h in range(8):
        r128[h, 16 * h:16 * (h + 1)] = 1
    consts['r128'] = r128.astype(bf)
    consts['ident'] = np.eye(128, dtype=f).astype(bf)
    return consts


def core_inputs(inputs, consts):
    """Build per-core in_maps (pad + slice on host)."""
    src = np.asarray(inputs['src'], np.float32)
    maps = []
    for core in range(NCORE):
        b = core // 2
        r0 = (core % 2) * RPC
        xp = np.zeros((C, R2, W2), np.float32)
        rlo, rhi = r0 - 1, r0 + RPC + 1
        slo, shi = max(rlo, 0), min(rhi, H)
        xp[:, (slo - rlo):(shi - rlo), 1:W + 1] = src[b, :, slo:shi, :]
        m = {'x': xp.reshape(C, NT)}
        m.update(consts)
        maps.append(m)
    return maps


def build(erep_dma=EREP_DMA):
    nc = bacc.Bacc('TRN2', target_bir_lowering=False, debug=False, num_devices=NCORE)

    x_d = nc.dram_tensor('x', [C, NT], F32, kind='ExternalInput')
    wt_d = nc.dram_tensor('wt', [C, 3 * C], BF16, kind='ExternalInput')
    rk2_d = nc.dram_tensor('rk2', [2, 3 * C], BF16, kind='ExternalInput')
    owt_d = nc.dram_tensor('owt', [C, C], BF16, kind='ExternalInput')
    obc_d = nc.dram_tensor('obc', [2, 128, 1], F32, kind='ExternalInput')
    ones1_d = nc.dram_tensor('ones1', [128, 1], BF16, kind='ExternalInput')
    onesr_d = nc.dram_tensor('onesr', [1, 128], BF16, kind='ExternalInput')
    onesnt_d = nc.dram_tensor('onesnt', [1, NT], BF16, kind='ExternalInput')
    epsb_d = nc.dram_tensor('epsb', [3, 1], F32, kind='ExternalInput')
    mk_d = nc.dram_tensor('mk', [128, 9 * 72], BF16, kind='ExternalInput')
    tm_d = nc.dram_tensor('tm', [72, 8], BF16, kind='ExternalInput')
    rp_d = nc.dram_tensor('rp', [72, 9 * 128], BF16, kind='ExternalInput')
    r128_d = nc.dram_tensor('r128', [8, 128], BF16, kind='ExternalInput')
    id_d = nc.dram_tensor('ident', [128, 128], BF16, kind='ExternalInput')

    out_d = nc.dram_tensor('out', [C, RPC * W], F32, kind='ExternalOutput')

    with tile.TileContext(nc) as tc, ExitStack() as ctx:
        ctx.enter_context(nc.allow_low_precision(reason='bf16 attention path'))
        P = ctx.enter_context(tc.tile_pool(name='persist', bufs=1))
        T3 = ctx.enter_context(tc.tile_pool(name='work', bufs=3))
        PS = ctx.enter_context(tc.tile_pool(name='psum', bufs=1, space='PSUM'))

        mm = lambda *a, **kw: nc.tensor.matmul(*a, **kw)

        # ---- input x first (both queues), then constants ----
        xsb = [P.tile([128, NT], F32, tag=f'xsb{c}', name=f'xsb{c}') for c in range(2)]
        ones1_s = P.tile([128, 1], BF16, tag='ones1')
        epsb_s = P.tile([3, 1], F32, tag='epsb')
        for s in range(3):
            nc.sync.dma_start(xsb[0][:, bass.ts(s, 2 * SS)], x_d[0:128, bass.ts(s, 2 * SS)])
            nc.scalar.dma_start(xsb[1][:, bass.ts(s, 2 * SS)], x_d[128:256, bass.ts(s, 2 * SS)])
            if s == 0:
                nc.sync.dma_start(ones1_s[:], ones1_d[:])
                nc.scalar.dma_start(epsb_s[:], epsb_d[:])
        wt_c = [P.tile([128, 3 * C], BF16, tag=f'wt{c}', name=f'wt{c}') for c in range(2)]
        nc.sync.dma_start(wt_c[0][:], wt_d[0:128, :])
        nc.scalar.dma_start(wt_c[1][:], wt_d[128:256, :])
        rk2_s = P.tile([2, 3 * C], BF16, tag='rk2')
        nc.sync.dma_start(rk2_s[:], rk2_d[:])
        onesr_s = P.tile([1, 128], BF16, tag='onesr')
        nc.scalar.dma_start(onesr_s[:], onesr_d[:])
        owt_c = [P.tile([128, C], BF16, tag=f'owt{c}', name=f'owt{c}') for c in range(2)]
        nc.sync.dma_start(owt_c[0][:], owt_d[0:128, :])
        nc.scalar.dma_start(owt_c[1][:], owt_d[128:256, :])
        obc_s = [P.tile([128, 1], F32, tag=f'obc{c}', name=f'obc{c}') for c in range(2)]
        nc.sync.dma_start(obc_s[0][:], obc_d[0])
        nc.scalar.dma_start(obc_s[1][:], obc_d[1])
        mk_s = P.tile([128, 9 * 72], AD, tag='mk')
        nc.sync.dma_start(mk_s[:], mk_d[:])
        tm_s = P.tile([72, 8], AD, tag='tm')
        nc.scalar.dma_start(tm_s[:], tm_d[:])
        rp_s = P.tile([72, 9 * 128], AD, tag='rp')
        nc.sync.dma_start(rp_s[:], rp_d[:])
        r128_s = P.tile([8, 128], AD, tag='r128')
        nc.scalar.dma_start(r128_s[:], r128_d[:])
        id_s = P.tile([128, 128], AD, tag='ident')
        nc.sync.dma_start(id_s[:], id_d[:])
        r2 = P.tile([2, NT], BF16, tag='r2')
        nc.scalar.dma_start(r2[1:2, :], onesnt_d[:])
        rstd1 = P.tile([1, NT], BF16, tag='rstd1')

        xb16 = [P.tile([128, NT], BF16, tag=f'xb16{c}', name=f'xb16{c}') for c in range(2)]
        xhat = [P.tile([128, NT], BF16, tag=f'xhat{c}', name=f'xhat{c}') for c in range(2)]
        qkv = [P.tile([128, 2, NT], AD, tag=f'qkv{t}', name=f'qkv{t}') for t in range(3)]

        # ---- per-strip: stats -> (batched LN) -> xhat -> qkv ----
        evac_rr = [0]

        def evac(dst_ap, src_ap):
            if evac_rr[0] % 2 == 0:
                nc.scalar.activation(dst_ap, src_ap, mybir.ActivationFunctionType.Copy)
            else:
                nc.vector.tensor_copy(dst_ap, src_ap)
            evac_rr[0] += 1

        NB = 3  # strips per LN batch
        mu_rows = {}
        for s in range(NS):
            sl = bass.ts(s, SS)
            b = s // NB
            if s % NB == 0:
                mu_rows[b] = (T3.tile([NB, SS], F32, tag='mu6', name='mu6', bufs=2),
                              T3.tile([NB, SS], F32, tag='m26', name='m26', bufs=2))
            mub, m2b = mu_rows[b]
            xsq = [T3.tile([128, SS], BF16, tag='xsq', name='xsq', bufs=4) for _ in range(2)]
            for c in range(2):
                nc.vector.tensor_copy(xb16[c][:, sl], xsb[c][:, sl])
                nc.scalar.square(xsq[c][:], xsb[c][:, sl])
            ps_sx = PS.tile([1, SS], F32, tag='psB', bufs=2, name='ps_sx')
            ps_sxx = PS.tile([1, SS], F32, tag='psB', bufs=2, name='ps_sxx')
            for c in range(2):
                mm(ps_sx[:], ones1_s[:], xb16[c][:, sl], start=(c == 0), stop=(c == 1))
                mm(ps_sxx[:], ones1_s[:], xsq[c][:], start=(c == 0), stop=(c == 1))
            tmu = T3.tile([1, SS], F32, tag='txe', name='tmu', bufs=4)
            tm2 = T3.tile([1, SS], F32, tag='txe', name='tm2', bufs=4)
            nc.scalar.activation(tmu[:], ps_sx[:],
                                 mybir.ActivationFunctionType.Copy, scale=1.0 / C)
            nc.vector.tensor_copy(tm2[:], ps_sxx[:])
            nc.sync.dma_start(mub[s % NB:s % NB + 1, :], tmu[:])
            nc.scalar.dma_start(m2b[s % NB:s % NB + 1, :], tm2[:])

            if s % NB == NB - 1:
                # batched LN math for strips [b*NB, b*NB+NB)
                bl = bass.ts(b, NB * SS)
                musq = T3.tile([NB, SS], F32, tag='musq', bufs=2)
                nc.vector.tensor_mul(musq[:], mub[:], mub[:])
                var = T3.tile([NB, SS], F32, tag='var', bufs=2)
                # var = m2/C - musq
                nc.vector.scalar_tensor_tensor(var[:], m2b[:], 1.0 / C, musq[:],
                                               mybir.AluOpType.mult,
                                               mybir.AluOpType.subtract)
                sd = T3.tile([NB, SS], F32, tag='sd', bufs=2)
                nc.scalar.activation(sd[:], var[:], mybir.ActivationFunctionType.Sqrt,
                                     bias=epsb_s[:])
                rstd = T3.tile([NB, SS], F32, tag='rstd', bufs=2)
                nc.vector.reciprocal_approx_fast(rstd[:], sd[:])
                mrs = T3.tile([NB, SS], BF16, tag='mrs', bufs=2)
                nc.vector.tensor_mul(mrs[:], mub[:], rstd[:])
                rsb = T3.tile([NB, SS], BF16, tag='rsb', bufs=2)
                nc.vector.tensor_copy(rsb[:], rstd[:])
                nc.sync.dma_start(r2[0:1, bl], mrs[:])
                nc.scalar.dma_start(rstd1[:, bl], rsb[:])
                strips = list(range(b * NB, b * NB + NB))
                for s2 in strips:
                    sl2 = bass.ts(s2, SS)
                    ps_rb = PS.tile([128, SS], F32, tag='psO1', bufs=1, name='ps_rb')
                    mm(ps_rb[:], onesr_s[:], rstd1[:, sl2], start=True, stop=True)
                    rstdb = T3.tile([128, SS], BF16, tag='rstdb', name='rstdb', bufs=2)
                    nc.scalar.activation(rstdb[:], ps_rb[:],
                                         mybir.ActivationFunctionType.Copy)
                    for c in range(2):
                        nc.vector.tensor_mul(xhat[c][:, sl2], xb16[c][:, sl2], rstdb[:])
                # qkv: per dblock, run each lhsT across all 3 strips back-to-back
                for d in range(6):
                    dl = bass.ts(d, 128)
                    tens, half = qkv[d // 2], d % 2
                    pqs = {}
                    for k, s2 in enumerate(strips):
                        pqs[s2] = PS.tile([128, SS], F32,
                                          tag=('psA' if k % 2 == 0 else 'psC'),
                                          bufs=2, name='pq')
                    for c in range(2):
                        for s2 in strips:
                            mm(pqs[s2][:], wt_c[c][:, dl], xhat[c][:, bass.ts(s2, SS)],
                               start=(c == 0), stop=False)
                    for s2 in strips:
                        mm(pqs[s2][:], rk2_s[:, dl], r2[:, bass.ts(s2, SS)],
                           start=False, stop=True)
                    for s2 in strips:
                        evac(tens[:, half, bass.ts(s2, SS)], pqs[s2][:])

        # views [128, 2, 34, 66]
        g = lambda tn: tn[:].rearrange('p h (r w) -> p h r w', w=W2)
        qg, kg, vg = g(qkv[0]), g(qkv[1]), g(qkv[2])
        xg = [xsb[c][:].rearrange('p (r w) -> p r w', w=W2) for c in range(2)]

        # ---- attention per core strip ----
        for cs in range(NCS):
            r0 = 1 + 8 * cs
            ps_sc = PS.tile([72, CS], F32, tag='psB', bufs=2, name='ps_sc')
            for t, (i, j) in enumerate(OFFS):
                prod = T3.tile([128, 2, 8, W], AD, tag='prod', bufs=4, name='prod')
                nc.vector.tensor_mul(
                    prod[:],
                    qg[:, :, r0:r0 + 8, 1:1 + W],
                    kg[:, :, r0 + i - 1:r0 + i + 7, j:j + W])
                for c in range(2):
                    mm(ps_sc[:, :], mk_s[:, bass.ts(t, 72)], prod[:, c],
                       start=(t == 0 and c == 0), stop=(t == 8 and c == 1))
            e_sb = T3.tile([72, CS], AD, tag='e_sb', bufs=2)
            nc.scalar.activation(e_sb[:], ps_sc[:], mybir.ActivationFunctionType.Exp)
            # denominator path runs concurrent with the erep/avp rounds below;
            # normalization is applied at o evac time
            ps_den = PS.tile([8, CS], F32, tag='psA', bufs=2, name='ps_den')
            mm(ps_den[:], tm_s[:], e_sb[:], start=True, stop=True)
            rdenf = T3.tile([8, CS], F32, tag='rdenf', bufs=2)
            nc.vector.reciprocal_approx_fast(rdenf[:], ps_den[:])
            rden = T3.tile([8, CS], AD, tag='rden', bufs=2)
            nc.vector.tensor_copy(rden[:], rdenf[:])
            ps_r72 = PS.tile([128, CS], F32, tag='psA', bufs=2, name='ps_r72')
            mm(ps_r72[:], r128_s[:], rden[:], start=True, stop=True)
            rdrep = T3.tile([128, CS], AD, tag='rdrep', bufs=2)
            nc.scalar.activation(rdrep[:], ps_r72[:], mybir.ActivationFunctionType.Copy)
            attn = e_sb

            o_ps = [PS.tile([128, CS], F32, tag=f'psO{c}', bufs=1, name=f'o_ps{c}')
                    for c in range(2)]
            for t, (i, j) in enumerate(OFFS):
                erep = T3.tile([128, CS], AD, tag='erep', bufs=10)
                if erep_dma:
                    src = attn[t * 8:(t + 1) * 8, :].unsqueeze(1).broadcast_to(
                        (8, 16, CS))
                    dst = erep[:].rearrange('(h d) w -> h d w', d=16)
                    (nc.sync if t % 2 == 0 else nc.scalar).dma_start(dst, src)
                else:
                    ps_er = PS.tile([128, CS], F32, tag='psC', bufs=2, name='ps_er')
                    mm(ps_er[:], rp_s[:, bass.ts(t, 128)], attn[:],
                       start=True, stop=True)
                    nc.scalar.activation(erep[:], ps_er[:],
                                         mybir.ActivationFunctionType.Copy)
                avp = T3.tile([128, 2, 8, W], AD, tag='avp', bufs=4, name='avp')
                erv = erep[:].rearrange('p (r w) -> p r w', w=W)
                for c in range(2):
                    nc.vector.tensor_mul(
                        avp[:, c], erv,
                        vg[:, c, r0 + i - 1:r0 + i + 7, j:j + W])
                for c in range(2):
                    mm(o_ps[c][:], id_s[:], avp[:, c], start=(t == 0), stop=(t == 8))
            o_sb = [T3.tile([128, CS], BF16, tag=f'o_sb{c}', name=f'o_sb{c}', bufs=2)
                    for c in range(2)]
            for c in range(2):
                nc.vector.tensor_mul(o_sb[c][:], o_ps[c][:], rdrep[:])

            # ---- out projection; bias + residual folded into evac ----
            for db in range(2):
                op_ps = PS.tile([128, CS], F32, tag='psA', bufs=2, name='op_ps')
                for c in range(2):
                    mm(op_ps[:], owt_c[c][:, bass.ts(db, 128)], o_sb[c][:],
                       start=(c == 0), stop=(c == 1))
                ot = T3.tile([128, 8, W], F32, tag='ot', bufs=2)
                nc.vector.scalar_tensor_tensor(
                    ot[:], op_ps[:].rearrange('p (r w) -> p r w', w=W),
                    obc_s[db][:], xg[db][:, r0:r0 + 8, 1:1 + W],
                    mybir.AluOpType.add, mybir.AluOpType.add)
                nc.sync.dma_start(
                    out_d[bass.ts(db, 128), bass.ts(cs, CS)], ot[:])

    nc.compile()
    return nc


_NC_CACHE = {}


def _get_nc(**kw):
    key = ('nc',) + tuple(sorted(kw.items()))
    if key not in _NC_CACHE:
        _NC_CACHE[key] = build(**kw)
    return _NC_CACHE[key]


def kernel(**inputs):
    """Full-input, full-output entry point. Shards over 8 NeuronCores."""
    from concourse.bass_utils import run_bass_kernel_spmd
    nc = _get_nc()
    consts = host_prep(inputs)
    maps = core_inputs(inputs, consts)
    res = run_bass_kernel_spmd(nc, maps, core_ids=list(range(NCORE)))
    out = np.zeros((B, C, H, W), np.float32)
    for core in range(NCORE):
        b = core // 2
        r0 = (core % 2) * RPC
        out[b, :, r0:r0 + RPC, :] = res.results[core]['out'].reshape(C, RPC, W)
    return out
